# revision 30
# baseline (speedup 1.0000x reference)
"""Trainium2 Bass kernel for an attentive LSTM cell.

Data-parallel across 8 NeuronCores: batch (64) is sharded 8 per core, all
weights replicated.  Per core, for each batch item the kernel streams the
[2048, 512] annotation matrix through SBUF in [512, 512] tiles:

  1. PE-transposes each tile block-wise (ann^T needed because the matmul
     contraction runs over the partition dim), staging in PSUM.
  2. uh^T = kernel_u^T @ ann^T accumulated in PSUM (float32r matmuls: full
     1 cycle/row rate at N=512, vs 4 cycles/row for plain fp32).
  3. tanh(uh + Wx + bias_u) fused on the scalar engine (per-partition bias).
  4. et = v . tanh(...) via a v-stationary matmul; exp on the scalar engine
     with the softmax denominator accumulated in the same instruction.
  5. context += w^T @ ann (natural-layout tile), normalized at the end.

The LSTM tail (z = x@W + h@R + b, gates, c/h update) runs batched over the
core's 8 rows with x^T/h^T assembled from tiny PE transposes.
"""

import os
import sys

for _p in ("/opt/trn_rl_repo", "/root/.axon_site/_ro/trn_rl_repo"):
    if os.path.isdir(_p) and _p not in sys.path:
        sys.path.insert(0, _p)

import numpy as np

import concourse.bass as bass
import concourse.mybir as mybir
import concourse.tile as tile
from concourse import bacc
from concourse.bass_utils import run_bass_kernel_spmd
from concourse.masks import make_identity

AF = mybir.ActivationFunctionType
F32 = mybir.dt.float32
F32R = mybir.dt.float32r
BF16 = mybir.dt.bfloat16
USE_BF16_ANN = True

N_CORES = 8
B, T, A, U, D = 64, 2048, 512, 512, 512
BS = B // N_CORES  # batch rows per core
TT = 512           # t macro-tile
NT = T // TT       # macro tiles per batch row
NS = TT // 128     # 128-row subtiles per macro tile
J = A // 128       # contraction chunks (annotation dim)
M = U // 128       # unit chunks


def _r(ap):
    return ap.bitcast(F32R)


def build_bass(stage="full", repeat=1):
    nc = bacc.Bacc(trn_type="TRN2", debug=False)

    ann_d = nc.dram_tensor("ann", [BS, T, A], F32, kind="ExternalInput").ap()
    inp_d = nc.dram_tensor("inputs", [BS, D], F32, kind="ExternalInput").ap()
    h_d = nc.dram_tensor("h", [BS, U], F32, kind="ExternalInput").ap()
    c_d = nc.dram_tensor("c", [BS, U], F32, kind="ExternalInput").ap()
    W_d = nc.dram_tensor("kernel", [D + A, 4 * U], F32, kind="ExternalInput").ap()
    R_d = nc.dram_tensor("rkernel", [U, 4 * U], F32, kind="ExternalInput").ap()
    bias_d = nc.dram_tensor("bias", [1, 6 * U], F32, kind="ExternalInput").ap()
    ku_d = nc.dram_tensor("ku", [A, U], F32, kind="ExternalInput").ap()
    kw_d = nc.dram_tensor("kw", [U, U], F32, kind="ExternalInput").ap()
    kv_d = nc.dram_tensor("kv", [1, U], F32, kind="ExternalInput").ap()
    out_d = nc.dram_tensor("out", [BS, U], F32, kind="ExternalOutput").ap()
    global _W_SCRATCH
    _W_SCRATCH = [nc.dram_tensor(f"wscratch{k}", [1, TT], F32R).ap()
                  for k in range(2)]

    with tile.TileContext(nc) as tc:
        if repeat > 1:
            with tc.For_i(0, repeat, 1):
                _body(nc, tc, ann_d, inp_d, h_d, c_d, W_d, R_d, bias_d, ku_d,
                      kw_d, kv_d, out_d, stage)
        else:
            _body(nc, tc, ann_d, inp_d, h_d, c_d, W_d, R_d, bias_d, ku_d,
                  kw_d, kv_d, out_d, stage)
    nc.compile()
    return nc


def _body(nc, tc, ann_d, inp_d, h_d, c_d, W_d, R_d, bias_d, ku_d, kw_d, kv_d,
          out_d, stage="full"):
    with (
        tc.tile_pool(name="const", bufs=1) as cpool,
        tc.tile_pool(name="wts", bufs=1) as wpool,
    ):
        ident = cpool.tile([128, 128], F32)
        make_identity(nc, ident)
        AT = BF16 if USE_BF16_ANN else F32R   # attention data dtype
        ident_t = cpool.tile([128, 128], BF16, name="ident_t") if USE_BF16_ANN else ident
        if USE_BF16_ANN:
            nc.vector.tensor_copy(ident_t, ident)
        ones11_t = cpool.tile([1, 1], BF16, name="ones11_t") if USE_BF16_ANN else None
        ident_r = cpool.tile([128, 128], F32R)
        nc.vector.tensor_copy(ident_r, ident)
        ones11 = cpool.tile([1, 1], F32)
        nc.vector.memset(ones11, 1.0)
        ones11_r = cpool.tile([1, 1], F32R)
        nc.vector.tensor_copy(ones11_r, ones11)
        if ones11_t is None:
            ones11_t = ones11_r
        else:
            nc.vector.tensor_copy(ones11_t, ones11)
        ones1b_ld = cpool.tile([1, BS], F32)
        nc.vector.memset(ones1b_ld, 1.0)
        ones1b = cpool.tile([1, BS], F32R)
        nc.vector.tensor_copy(ones1b, ones1b_ld)
        half_col = cpool.tile([BS, 1], F32)
        nc.vector.memset(half_col, 0.5)

        # --- replicated weights ---
        ku_ld = wpool.tile([128, J, U], F32)   # ku[a, u] -> [p, j, u], a=128j+p
        nc.sync.dma_start(out=ku_ld, in_=ku_d.rearrange("(j p) u -> p j u", p=128))
        ku_sb = wpool.tile([128, J, U], AT)
        nc.vector.tensor_copy(ku_sb, ku_ld)
        kw_ld = wpool.tile([128, J, U], F32)
        nc.sync.dma_start(out=kw_ld, in_=kw_d.rearrange("(j p) u -> p j u", p=128))
        kw_sb = wpool.tile([128, J, U], F32R)
        nc.vector.tensor_copy(kw_sb, kw_ld)
        v_ld = cpool.tile([128, M], F32)       # v[u] -> [p, m], u=128m+p
        nc.sync.dma_start(out=v_ld, in_=kv_d.rearrange("o (m p) -> p (o m)", p=128))
        v_col = cpool.tile([128, M], F32R)
        nc.vector.tensor_copy(v_col, v_ld)
        biasu_col = cpool.tile([128, M], F32)  # bias[4U:5U] as a column
        nc.sync.dma_start(
            out=biasu_col,
            in_=bias_d[:, 4 * U:5 * U].rearrange("o (m p) -> p (o m)", p=128))
        biasz_ld = cpool.tile([1, 4 * U], F32)
        nc.sync.dma_start(out=biasz_ld, in_=bias_d[:, 0:4 * U])
        biasz_row = cpool.tile([1, 4 * U], F32R)
        nc.vector.tensor_copy(biasz_row, biasz_ld)

        # --- per-core state rows ---
        h_nat = cpool.tile([BS, U], F32)
        nc.sync.dma_start(out=h_nat, in_=h_d)
        in_nat = cpool.tile([BS, D], F32)
        nc.sync.dma_start(out=in_nat, in_=inp_d)
        c_nat = cpool.tile([BS, U], F32)
        nc.sync.dma_start(out=c_nat, in_=c_d)

        hT = wpool.tile([128, M, BS], F32R)     # h^T, contraction layout
        xT = wpool.tile([128, 2 * J, BS], F32R)  # [inputs; context]^T
        bias_att = wpool.tile([128, M, BS], F32)  # Wx^T + bias_u per batch row

        with tc.tile_pool(name="ps_setup", bufs=2, space="PSUM") as pps:
            for j in range(M):
                pt = pps.tile([128, BS], F32)
                nc.tensor.transpose(pt, h_nat[:, 128 * j:128 * (j + 1)],
                                    ident[0:BS, 0:BS])
                nc.vector.tensor_copy(hT[:, j, :], pt)
            for j in range(J):
                pt = pps.tile([128, BS], F32)
                nc.tensor.transpose(pt, in_nat[:, 128 * j:128 * (j + 1)],
                                    ident[0:BS, 0:BS])
                nc.vector.tensor_copy(xT[:, j, :], pt)
            for m in range(M):
                pwx = pps.tile([128, BS], F32)
                for j in range(M):
                    nc.tensor.matmul(pwx,
                                     lhsT=kw_sb[:, j, 128 * m:128 * (m + 1)],
                                     rhs=hT[:, j, :],
                                     start=(j == 0), stop=(j == M - 1))
                nc.scalar.activation(bias_att[:, m, :], pwx, AF.Identity,
                                     bias=biasu_col[:, m:m + 1])

        dump = cpool.tile([BS, U], F32)
        nc.vector.memset(dump, 0.0)
        if stage == "setup":
            nc.vector.tensor_copy(dump[:, 0:BS], hT[0:BS, 0, :])
            nc.vector.tensor_copy(dump[:, BS:2 * BS], bias_att[0:BS, 0, :])
            nc.sync.dma_start(out=out_d, in_=dump)
            return

        # ------------- attention over the annotation stream -------------
        with (
            tc.tile_pool(name="ann", bufs=2) as annpool,
            tc.tile_pool(name="annT", bufs=2) as annTpool,
            tc.tile_pool(name="tanh", bufs=2) as tanhpool,
            tc.tile_pool(name="big_ps", bufs=3, space="PSUM") as bigps,
            tc.tile_pool(name="small_ps", bufs=2, space="PSUM") as smallps,
            tc.tile_pool(name="small_sb", bufs=2) as smallsb,
        ):
            for b in range(BS):
                ctx_acc = smallsb.tile([1, A], F32, tag="ctxacc")
                nc.vector.memset(ctx_acc, 0.0)
                denb = smallsb.tile([1, NT], F32, tag="den")
                for i in range(NT):
                    ann_t = annpool.tile([128, NS, A], F32)
                    nc.sync.dma_start(
                        out=ann_t,
                        in_=ann_d[b, TT * i:TT * (i + 1), :].rearrange(
                            "(s p) a -> p s a", p=128))
                    ann_r = annpool.tile([128, NS, A], AT, tag="ann_r")
                    nc.vector.tensor_copy(ann_r, ann_t)

                    annT = annTpool.tile([128, J, TT], AT)
                    tr_src = ann_r if USE_BF16_ANN else ann_t
                    tr_id = ident_t if USE_BF16_ANN else ident
                    stg_dt = BF16 if USE_BF16_ANN else F32
                    for j in range(J):
                        stg = bigps.tile([128, TT], stg_dt, tag="big")
                        for s in range(NS):
                            nc.tensor.transpose(
                                stg[:, 128 * s:128 * (s + 1)],
                                tr_src[:, s, 128 * j:128 * (j + 1)], tr_id)
                        nc.vector.tensor_copy(annT[:, j, :], stg)
                    if stage == "transp":
                        nc.vector.tensor_copy(dump, annT[0:BS, 0, :])
                        continue

                    tanhG = tanhpool.tile([128, M, TT], F32R)
                    for mg in range(M // 2):
                        gps = bigps.tile([128, 2, TT], F32, tag="big")
                        for mi in range(2):
                            m = 2 * mg + mi
                            for j in range(J):
                                nc.tensor.matmul(
                                    gps[:, mi, :],
                                    lhsT=ku_sb[:, j, 128 * m:128 * (m + 1)],
                                    rhs=annT[:, j, :],
                                    start=(j == 0), stop=(j == J - 1))
                            nc.scalar.activation(tanhG[:, m, :], gps[:, mi, :],
                                                 AF.Tanh,
                                                 bias=bias_att[:, m, b:b + 1])

                    if stage == "g":
                        nc.vector.tensor_copy(dump, tanhG[0:BS, 0, :])
                        continue

                    et_ps = smallps.tile([1, TT], F32, tag="sm")
                    for m in range(M):
                        nc.tensor.matmul(et_ps, lhsT=v_col[:, m:m + 1],
                                         rhs=tanhG[:, m, :],
                                         start=(m == 0), stop=(m == M - 1))
                    w_row = smallsb.tile([1, TT], AT, tag="wrow")
                    nc.scalar.activation(w_row, et_ps, AF.Exp,
                                         accum_out=denb[:, i:i + 1])

                    wcw = 2 if USE_BF16_ANN else 1  # pad bf16 cols to 4B
                    wc_ps = smallps.tile([128, NS * wcw], AT, tag="sm")
                    for s in range(NS):
                        nc.tensor.transpose(wc_ps[:, wcw * s:wcw * s + 1],
                                            w_row[:, 128 * s:128 * (s + 1)],
                                            ones11_t if USE_BF16_ANN else ones11_r)
                    w_col = smallsb.tile([128, NS], AT, tag="wcol")
                    if USE_BF16_ANN:
                        nc.vector.tensor_copy(
                            w_col, wc_ps.rearrange("p (s w) -> p s w", w=2)[:, :, 0])
                    else:
                        nc.vector.tensor_copy(w_col, wc_ps)

                    if stage == "et":
                        nc.vector.tensor_copy(dump[0:1, :], w_row)
                        continue

                    ctx_ps = smallps.tile([1, A], F32, tag="sm")
                    for s in range(NS):
                        nc.tensor.matmul(ctx_ps, lhsT=w_col[:, s:s + 1],
                                         rhs=ann_r[:, s, :],
                                         start=(s == 0), stop=(s == NS - 1))
                    nc.vector.tensor_add(ctx_acc, ctx_acc, ctx_ps)

                if stage in ("transp", "g", "et"):
                    continue
                # normalize context, transpose into xT[:, J:2J, b]
                dsum = smallsb.tile([1, 1], F32, tag="dsum")
                nc.vector.reduce_sum(dsum, denb, axis=mybir.AxisListType.X)
                drec = smallsb.tile([1, 1], F32, tag="drec")
                nc.vector.reciprocal(drec, dsum)
                ctx_row = smallsb.tile([1, A], F32, tag="ctxrow")
                nc.vector.tensor_scalar_mul(ctx_row, ctx_acc, drec)
                cT_ps = smallps.tile([128, J], F32, tag="sm")
                for j in range(J):
                    nc.tensor.transpose(cT_ps[:, j:j + 1],
                                        ctx_row[:, 128 * j:128 * (j + 1)],
                                        ones11)
                nc.vector.tensor_copy(xT[:, J:2 * J, b], cT_ps)
                if stage == "ctx":
                    nc.vector.tensor_copy(dump[0:1, :], ctx_row)

        if stage in ("transp", "g", "et", "ctx"):
            nc.sync.dma_start(out=out_d, in_=dump)
            return

        # ------------- LSTM tail, batched over the core's rows -------------
        with (
            tc.tile_pool(name="wstream", bufs=2) as wsp,
            tc.tile_pool(name="z_ps", bufs=2, space="PSUM") as zpool,
            tc.tile_pool(name="gates", bufs=1) as gpool,
        ):
            gates = []
            for n in range(4):
                Wn_ld = wsp.tile([128, 2 * J, U], F32, tag="wn_ld")
                nc.sync.dma_start(
                    out=Wn_ld,
                    in_=W_d[:, U * n:U * (n + 1)].rearrange(
                        "(k p) n -> p k n", p=128))
                Wn = wsp.tile([128, 2 * J, U], F32R, tag="wn")
                nc.vector.tensor_copy(Wn, Wn_ld)
                Rn_ld = wsp.tile([128, M, U], F32, tag="rn_ld")
                nc.sync.dma_start(
                    out=Rn_ld,
                    in_=R_d[:, U * n:U * (n + 1)].rearrange(
                        "(k p) n -> p k n", p=128))
                Rn = wsp.tile([128, M, U], F32R, tag="rn")
                nc.vector.tensor_copy(Rn, Rn_ld)
                zps = zpool.tile([BS, U], F32)
                for k in range(2 * J):
                    nc.tensor.matmul(zps, lhsT=xT[:, k, :],
                                     rhs=Wn[:, k, :],
                                     start=(k == 0), stop=False)
                for k in range(M):
                    nc.tensor.matmul(zps, lhsT=hT[:, k, :],
                                     rhs=Rn[:, k, :],
                                     start=False, stop=False)
                nc.tensor.matmul(zps, lhsT=ones1b,
                                 rhs=biasz_row[:, U * n:U * (n + 1)],
                                 start=False, stop=True)
                g = gpool.tile([BS, U], F32, tag=f"gate{n}")
                if n == 2:  # candidate cell state
                    nc.scalar.activation(g, zps, AF.Tanh)
                else:       # hard sigmoid: clip(0.2 z + 0.5, 0, 1)
                    nc.scalar.activation(g, zps, AF.Relu, bias=half_col,
                                         scale=0.2)
                    nc.vector.tensor_scalar_min(g, g, 1.0)
                gates.append(g)

            gi, gf, gg, go = gates
            c_new = gpool.tile([BS, U], F32, tag="cnew")
            nc.vector.tensor_mul(c_new, gf, c_nat)
            ig = gpool.tile([BS, U], F32, tag="ig")
            nc.vector.tensor_mul(ig, gi, gg)
            nc.vector.tensor_add(c_new, c_new, ig)
            tc_t = gpool.tile([BS, U], F32, tag="tanhc")
            nc.scalar.activation(tc_t, c_new, AF.Tanh)
            h_new = gpool.tile([BS, U], F32, tag="hnew")
            nc.vector.tensor_mul(h_new, go, tc_t)
            nc.sync.dma_start(out=out_d, in_=h_new)


_NC_CACHE = None


def _get_nc():
    global _NC_CACHE
    if _NC_CACHE is None:
        _NC_CACHE = build_bass()
    return _NC_CACHE


def make_in_maps(inputs, h, c, annotations, kernel, recurrent_kernel, bias,
                 kernel_u, kernel_w, kernel_v):
    asc = np.ascontiguousarray
    maps = []
    for core in range(N_CORES):
        sl = slice(core * BS, (core + 1) * BS)
        maps.append({
            "ann": asc(annotations[sl]).astype(np.float32),
            "inputs": asc(inputs[sl]).astype(np.float32),
            "h": asc(h[sl]).astype(np.float32),
            "c": asc(c[sl]).astype(np.float32),
            "kernel": asc(kernel).astype(np.float32),
            "rkernel": asc(recurrent_kernel).astype(np.float32),
            "bias": asc(bias).reshape(1, 6 * U).astype(np.float32),
            "ku": asc(kernel_u).astype(np.float32),
            "kw": asc(kernel_w).astype(np.float32),
            "kv": asc(kernel_v).reshape(1, U).astype(np.float32),
        })
    return maps


def kernel(inputs, h, c, annotations, kernel, recurrent_kernel, bias,
           kernel_u, kernel_w, kernel_v, _trace=False):
    nc = _get_nc()
    in_maps = make_in_maps(inputs, h, c, annotations, kernel,
                           recurrent_kernel, bias, kernel_u, kernel_w,
                           kernel_v)
    res = run_bass_kernel_spmd(nc, in_maps, list(range(N_CORES)),
                               trace=_trace)
    out = np.concatenate([res.results[i]["out"] for i in range(N_CORES)],
                         axis=0)
    if _trace:
        kernel.last_exec_time_ns = res.exec_time_ns
        kernel.last_results = res
    return out
